# revision 83
# baseline (speedup 1.0000x reference)
"""Trainium2 Bass kernel for nn_MultiHeadAttention_8074538516581.

Sharding: 8 cores = batch(4) x head-group(2 groups of 6 heads).
Each core computes, for its (b, g): qkv slice projection (bf16 matmuls,
fp32 psum accum; the struct-embed term is pre-added into x on the host in
fp32), per-head attention with the reference's exact semantics (q/k rounded
to bf16, shift-free softmax -- the row-max subtraction cancels in the
normalization; the [-30,30] logit clip and the 1e5/1e-10 guards are
provably inactive here), and the partial output projection over its 384
head-dims.  Host sums the two head-group partials per batch and adds b_out.

Token permutation: queries with (t % 64) % 3 == 0 are zeroed by the
reference's load mask, making their attention output mean(v) per head.
Tokens are permuted live-first so the 672 live queries are contiguous:
scores/exp run only on live columns plus one pinned-zero query column
whose output is exactly mean(v); that column's value is broadcast to the
other 351 masked tokens.

Attention layout: scores/probabilities live as [k_tok, q_tok]; the pv
matmul runs "transposed" (stationary = p chunk, moving = [v | 1]) so each
accumulation step streams only 65 columns and the softmax denominator
lands as column 64 of the per-(head, q-chunk) accumulator.  Normalization
is then a per-partition reciprocal+multiply on DVE, and attnT (needed by
the output projection) is produced by XBAR DMA transposes.
"""
import numpy as np
import ml_dtypes

import concourse.bass as bass
import concourse.mybir as mybir
import concourse.tile as tile
from concourse import bacc
from concourse.bass import ts
from concourse.bass_utils import run_bass_kernel_spmd

B, T, E = 4, 1024, 768
H, D = 12, 64
HG = 6                  # heads per group
GD = HG * D             # 384 head-dims per group
BLOCK_M = 64
LIVE = 672              # tokens with (t % BLOCK_M) % 3 != 0
MASK = T - LIVE         # 352
NQ = LIVE + 1           # live queries + one pinned-zero query (= mean-v row)
SCALE = 1.0 / 8.0       # 1/sqrt(64)
QC = 6                  # q chunks of 128 (last one holds 33 live+pinned cols)

BF16 = mybir.dt.bfloat16
F32 = mybir.dt.float32

_perm = None
_nc = None

TRACE = False
LAST_RES = None


def _perm_live_first():
    t = np.arange(T)
    m = (t % BLOCK_M) % 3 == 0
    return np.concatenate([t[~m], t[m]])


def _build_bass(debug=False):
    nc = bacc.Bacc()
    xT_d = nc.dram_tensor("xT", [E, T], BF16, kind="ExternalInput")
    wT_d = nc.dram_tensor("wT", [E, 3 * GD], BF16, kind="ExternalInput")
    woT_d = nc.dram_tensor("woT", [GD, E], BF16, kind="ExternalInput")
    out_d = nc.dram_tensor("out", [T, E], BF16, kind="ExternalOutput")
    if debug:
        dbg = {nm: nc.dram_tensor(nm, sh, dt, kind="ExternalOutput")
               for nm, sh, dt in (
                   ("dbg_q", [128, 3, NQ], BF16),
                   ("dbg_k", [128, 3, T], BF16),
                   ("dbg_v", [128, 8, HG * (D + 1)], BF16),
                   ("dbg_pp", [128, 768], BF16),
                   ("dbg_acc", [128, QC * (D + 1)], F32),
                   ("dbg_att", [128, QC * 128], BF16),
                   ("dbg_attnT", [128, 3, T], BF16),
               )}

    xT_r = xT_d[:, :].rearrange("(c p) t -> p c t", p=128)
    wT_r = wT_d[:, :].rearrange("(c p) n -> p c n", p=128)

    with tile.TileContext(nc) as tc:
        with tc.tile_pool(name="singles", bufs=1) as S:
            xT_sb = S.tile([128, 6, T], BF16)
            wqk_sb = S.tile([128, 6, 2 * GD], BF16)
            wv_sb = S.tile([128, 6, GD], BF16)
            woT_sb = S.tile([128, 3, E], BF16)
            warm_sb = S.tile([128, 512], BF16)
            qT_sb = S.tile([128, 3, NQ], BF16)
            kT_sb = S.tile([128, 3, T], BF16)
            v_sb = S.tile([128, 8, HG * (D + 1)], BF16)   # per-head [v | 1]
            attnT_sb = S.tile([128, 3, T], BF16)
            dpre_sb = S.tile([1, 1], F32)
            ppb = [S.tile([128, 768], BF16, tag=f"ppb{j}", name=f"ppb{j}")
                   for j in range(5)]

            # ---- input DMAs, chunked so compute starts early
            nc.sync.dma_start(out=xT_sb[:, 0:2, :], in_=xT_r[:, 0:2, :])
            nc.sync.dma_start(out=wv_sb[:, 0:2, :],
                              in_=wT_r[:, 0:2, 2 * GD:3 * GD])
            nc.sync.dma_start(out=xT_sb[:, 2:4, :], in_=xT_r[:, 2:4, :])
            nc.sync.dma_start(out=wv_sb[:, 2:6, :],
                              in_=wT_r[:, 2:6, 2 * GD:3 * GD])
            nc.sync.dma_start(out=xT_sb[:, 4:5, :], in_=xT_r[:, 4:5, :])
            nc.sync.dma_start(out=xT_sb[:, 5:6, :], in_=xT_r[:, 5:6, :])
            nc.sync.dma_start(out=wqk_sb[:, :, 0:128], in_=wT_r[:, :, 0:128])
            nc.sync.dma_start(out=wqk_sb[:, :, GD:GD + 128],
                              in_=wT_r[:, :, GD:GD + 128])
            nc.sync.dma_start(out=wqk_sb[:, :, 128:GD],
                              in_=wT_r[:, :, 128:GD])
            nc.sync.dma_start(out=wqk_sb[:, :, GD + 128:2 * GD],
                              in_=wT_r[:, :, GD + 128:2 * GD])
            nc.sync.dma_start(out=woT_sb,
                              in_=woT_d[:, :].rearrange("(c p) n -> p c n", p=128))

            nc.vector.memset(warm_sb, 0.5)
            for j in range(5):
                # pad cols so the last q-chunk's [128,128] stationary reads
                # defined small values (keeps denominators finite)
                nc.vector.memset(ppb[j][:, NQ:768], 1e-10)
            v_ones = v_sb[:, :, :].rearrange("p a (h e) -> p a h e", e=D + 1)[:, :, :, D:D + 1]
            nc.vector.memset(v_ones, 1.0)
            # pinned-zero query column (mean-v row for masked tokens)
            nc.vector.memset(qT_sb[:, :, LIVE:NQ], 0.0)
            # preload the exp table while DMAs run
            nc.scalar.activation(dpre_sb, warm_sb[0:1, 0:1],
                                 mybir.ActivationFunctionType.Exp)

            # ---- Phase V + QK + attention.  V projects through 6 psum
            # banks (tts 0-5 pass-major, tts 6-7 tt-major on reused banks)
            # so the first qk slices can grab banks while V still runs.
            from contextlib import ExitStack
            with tc.tile_pool(name="ps_qk", bufs=2, space="PSUM") as ps_qk, \
                 tc.tile_pool(name="att", bufs=2) as att_pool, \
                 tc.tile_pool(name="rq", bufs=2) as rq_pool, \
                 ExitStack() as vstack:
                vp = vstack.enter_context(
                    tc.tile_pool(name="vps", bufs=1, space="PSUM"))

                ppi = 0

                def qk_slice(c, which, s0, s1, j):
                    # one 1-bank slice of the q or k projection for pair c
                    qp = ps_qk.tile([128, 512], F32, tag="qk",
                                    name=f"qk{c}_{j}")
                    wof = c * 128 if which == "q" else GD + c * 128
                    dstT = qT_sb if which == "q" else kT_sb
                    for ek in range(6):
                        nc.tensor.matmul(qp[:, 0:s1 - s0],
                                         wqk_sb[:, ek, wof:wof + 128],
                                         xT_sb[:, ek, s0:s1],
                                         start=(ek == 0), stop=(ek == 5))
                    nc.vector.tensor_copy(dstT[:, c, s0:s1], qp[:, 0:s1 - s0])

                vt = [vp.tile([128, GD], F32, tag=f"v{tt % 6}", name=f"vt{tt}")
                      for tt in range(6)]
                for i in range(5):
                    nc.tensor.matmul(vt[0], warm_sb[:, 0:128], warm_sb[:, 0:384],
                                     start=True, stop=True)

                def vcopy(tt, t):
                    dst = v_sb[:, tt, :].rearrange(
                        "p (h e) -> p h e", e=D + 1)[:, :, 0:D]
                    src = t[:, :].rearrange("p (h d) -> p h d", d=D)
                    if tt % 2 == 0:
                        nc.vector.tensor_copy(dst, src)
                    else:
                        nc.scalar.copy(dst, src)

                for gi, grp in enumerate(((0, 1), (2, 3), (4,), (5,))):
                    for tt in range(6):
                        for ek in grp:
                            nc.tensor.matmul(vt[tt],
                                             xT_sb[:, ek, ts(tt, 128)],
                                             wv_sb[:, ek, :],
                                             start=(ek == 0), stop=(ek == 5))
                        if grp[-1] == 5:
                            vcopy(tt, vt[tt])
                    if gi == 1:
                        qk_slice(0, "q", 0, 512, 0)
                    elif gi == 2:
                        qk_slice(0, "k", 0, 512, 1)
                for tt in (6, 7):
                    t2 = vp.tile([128, GD], F32, tag=f"v{tt % 6}",
                                 name=f"vt{tt}")
                    for ek in range(6):
                        nc.tensor.matmul(t2, xT_sb[:, ek, ts(tt, 128)],
                                         wv_sb[:, ek, :],
                                         start=(ek == 0), stop=(ek == 5))
                    vcopy(tt, t2)
                    if tt == 6:
                        qk_slice(0, "q", 512, LIVE, 2)
                vstack.close()
                ps_s = vstack.enter_context(
                    tc.tile_pool(name="ps_s", bufs=2, space="PSUM"))
                ps_acc = vstack.enter_context(
                    tc.tile_pool(name="ps_acc", bufs=1, space="PSUM"))
                for c in range(3):
                    accs = [ps_acc.tile([128, QC * (D + 1)], F32,
                                        tag=f"acc{i}", name=f"acc{c}_{i}")
                            for i in range(2)]

                    def pv(kt, i, pp):
                        h = 2 * c + i
                        vh = v_sb[:, kt, h * (D + 1):(h + 1) * (D + 1)]
                        for qc in range(QC):
                            nc.tensor.matmul(
                                accs[i][:, qc * (D + 1):(qc + 1) * (D + 1)],
                                pp[:, qc * 128:(qc + 1) * 128],
                                vh,
                                # exactly one start=True per psum bank: it
                                # clears the whole bank's has_written, so the
                                # other regions' first writes (start=False on
                                # cleared bits) overwrite rather than add
                                start=(kt == 0 and qc == 0), stop=(kt == 7),
                                skip_group_check=True)

                    pend = []
                    for kt in range(8):
                        for i in range(2):      # head 2c+i
                            po = i * 64
                            kh = kT_sb[po:po + 64, c, ts(kt, 128)]
                            qh = qT_sb[po:po + 64, c, :]
                            sp = ps_s.tile([128, T], F32, tag="s",
                                           name=f"s{c}_{kt}_{i}")
                            nc.tensor.matmul(sp[:, 0:512], kh, qh[:, 0:512],
                                             start=True, stop=True)
                            nc.tensor.matmul(sp[:, 512:NQ], kh, qh[:, 512:NQ],
                                             start=True, stop=True)
                            pp = ppb[ppi % 5]
                            ppi += 1
                            nc.scalar.activation(pp[:, 0:NQ], sp[:, 0:NQ],
                                                 mybir.ActivationFunctionType.Exp,
                                                 scale=SCALE)
                            if debug and c == 0 and kt == 0 and i == 0:
                                nc.sync.dma_start(out=dbg["dbg_pp"][:, :], in_=pp)
                            pend.append((kt, i, pp))
                        # pv runs one kt behind exp so no PE instruction ever
                        # sits in the wait queue blocking later scores
                        while len(pend) > 3:
                            pv(*pend.pop(0))
                        if kt == 0 and c == 0:
                            qk_slice(0, "k", 512, T, 3)
                        if 2 <= kt <= 5 and c < 2:
                            j = kt - 2
                            which, s0, s1 = (("q", 0, 512), ("k", 0, 512),
                                             ("q", 512, LIVE), ("k", 512, T))[j]
                            qk_slice(c + 1, which, s0, s1, j)
                    # ---- normalize: per-partition recip + strided multiply
                    att = att_pool.tile([128, QC * 128], BF16, tag="att")
                    def norm(i):
                        rq = rq_pool.tile([128, QC], F32, tag=f"rq{i}")
                        den = bass.AP(tensor=accs[i].tensor, offset=accs[i].offset + D,
                                      ap=[list(accs[i].ap[0])] + [[D + 1, QC]])
                        nc.vector.reciprocal(rq, den)
                        src = bass.AP(tensor=accs[i].tensor, offset=accs[i].offset,
                                      ap=[list(accs[i].ap[0])] + [[D + 1, QC], [1, D]])
                        sca = bass.AP(tensor=rq.tensor, offset=rq.offset,
                                      ap=[list(rq.ap[0])] + [[1, QC], [0, D]])
                        dst = bass.AP(tensor=att.tensor, offset=att.offset + i * 64,
                                      ap=[list(att.ap[0])] + [[128, QC], [1, D]])
                        nc.vector.tensor_mul(dst, src, sca)
                    while pend:
                        kt_, i_, pp_ = pend.pop(0)
                        pv(kt_, i_, pp_)
                        if not any(e[1] == i_ for e in pend):
                            norm(i_)
                    # ---- transpose -> attnT [dims, tok].  c0/c1 ride the
                    # idle DMA engines (XBAR); c2 is on the critical tail, so
                    # use PE transposes + an ACT copy (no DMA latency) into a
                    # separate tile (keeps out-proj deps on c0/c1 precise)
                    o = attnT_sb[:, c, :]
                    if c < 2:
                        o3 = bass.AP(tensor=o.tensor, offset=o.offset,
                                     ap=[list(o.ap[0])] + [[128, QC], [1, 128]])
                        nc.sync.dma_start_transpose(o3, att[:, :])
                    else:
                        for h0 in (0, 3):
                            oh = bass.AP(tensor=o.tensor,
                                         offset=o.offset + h0 * 128,
                                         ap=[list(o.ap[0])] + [[128, 3], [1, 128]])
                            nc.sync.dma_start_transpose(
                                oh, att[:, h0 * 128:(h0 + 3) * 128])
                    if debug and c == 0:
                        accc = S.tile([128, QC * (D + 1)], F32, name="accc")
                        nc.vector.tensor_copy(accc, accs[0])
                        nc.sync.dma_start(out=dbg["dbg_acc"][:, :], in_=accc)
                        nc.sync.dma_start(out=dbg["dbg_att"][:, :], in_=att)

                if debug:
                    for nm, t in (("dbg_q", qT_sb), ("dbg_k", kT_sb),
                                  ("dbg_v", v_sb), ("dbg_attnT", attnT_sb)):
                        nc.sync.dma_start(out=dbg[nm][:, :, :], in_=t[:, :, :])

            # ---- Phase OUT: output projection (partial over group's dims);
            # one whole-tile staging copy per tt, alternating DVE/ACT
            with tc.tile_pool(name="ops", bufs=4, space="PSUM") as op, \
                 tc.tile_pool(name="ob", bufs=4) as obp:
                for tt in range(6):
                    ps = op.tile([128, E], F32, tag="o")
                    for s0, s1 in ((0, 512), (512, E)):
                        for c3 in range(3):
                            nc.tensor.matmul(ps[:, s0:s1],
                                             attnT_sb[:, c3, ts(tt, 128)],
                                             woT_sb[:, c3, s0:s1],
                                             start=(c3 == 0), stop=(c3 == 2))
                    if tt >= 4:
                        ob1 = obp.tile([128, E], BF16, tag="ob1",
                                       name=f"ob1_{tt}")
                        if tt == 5:
                            # rows 673:768 are masked duplicates the host
                            # rebuilds; ship only the 33 rows that matter
                            nc.scalar.copy(ob1[0:33, :], ps[0:33, :])
                            nc.sync.dma_start(out=out_d[640:673, :],
                                              in_=ob1[0:33, :])
                        else:
                            nc.vector.tensor_copy(ob1, ps)
                            nc.sync.dma_start(out=out_d[ts(tt, 128), :],
                                              in_=ob1)
                    elif tt % 2 == 0:
                        ob = obp.tile([128, 2, E], BF16, tag="ob")
                        nc.vector.tensor_copy(ob[:, 0, :], ps)
                    else:
                        nc.scalar.copy(ob[:, 1, :], ps)
                        nc.sync.dma_start(
                            out=out_d[(tt - 1) * 128:(tt + 1) * 128, :]
                            .rearrange("(j p) e -> p j e", p=128),
                            in_=ob)

    nc.finalize()
    return nc


def _get_bass():
    global _nc
    if _nc is None:
        _nc = _build_bass()
    return _nc


def kernel(x, idx, struct_embed, w_qkv, w_out, b_out):
    global _perm
    if _perm is None:
        _perm = _perm_live_first()
    perm = _perm

    x = np.asarray(x, dtype=np.float32)
    idx = np.asarray(idx)
    struct_embed = np.asarray(struct_embed, dtype=np.float32)
    w_qkv = np.asarray(w_qkv, dtype=np.float32)
    w_out = np.asarray(w_out, dtype=np.float32)
    b_out = np.asarray(b_out, dtype=np.float32)

    sid = ((idx == 1) * 1 + (idx == 2) * 2 + (idx == 3) * 3)  # [B,T]
    xs = x + struct_embed[sid]                                # fp32 pre-add

    bf = ml_dtypes.bfloat16
    in_maps = []
    for core in range(8):
        b, g = core // 2, core % 2
        wg = np.concatenate([w_qkv[g * GD:(g + 1) * GD],
                             w_qkv[E + g * GD:E + (g + 1) * GD],
                             w_qkv[2 * E + g * GD:2 * E + (g + 1) * GD]],
                            axis=0)  # [3GD, E]
        in_maps.append({
            "xT": np.ascontiguousarray(xs[b].T[:, perm]).astype(bf),
            "wT": np.ascontiguousarray(wg.T).astype(bf),
            "woT": np.ascontiguousarray(w_out[:, g * GD:(g + 1) * GD].T).astype(bf),
        })

    res = run_bass_kernel_spmd(_get_bass(), in_maps, core_ids=list(range(8)),
                               trace=TRACE)
    if TRACE:
        global LAST_RES
        LAST_RES = res

    inv = np.empty(T, dtype=np.int64)
    inv[perm] = np.arange(T)
    out = np.empty((B, T, E), dtype=np.float32)
    for b in range(B):
        acc = (res.results[2 * b]["out"].astype(np.float32)
               + res.results[2 * b + 1]["out"].astype(np.float32))
        # device computes permuted rows 0:NQ; all masked-token rows are
        # identical to row LIVE (the pinned-zero query = mean-v output)
        acc[NQ:] = acc[LIVE]
        out[b] = acc[inv] + b_out[None, :]
    return out


# revision 85
# speedup vs baseline: 1.0007x; 1.0007x over previous
"""Trainium2 Bass kernel for nn_MultiHeadAttention_8074538516581.

Sharding: 8 cores = batch(4) x head-group(2 groups of 6 heads).
Each core computes, for its (b, g): qkv slice projection (bf16 matmuls,
fp32 psum accum; the struct-embed term is pre-added into x on the host in
fp32), per-head attention with the reference's exact semantics (q/k rounded
to bf16, shift-free softmax -- the row-max subtraction cancels in the
normalization; the [-30,30] logit clip and the 1e5/1e-10 guards are
provably inactive here), and the partial output projection over its 384
head-dims.  Host sums the two head-group partials per batch and adds b_out.

Token permutation: queries with (t % 64) % 3 == 0 are zeroed by the
reference's load mask, making their attention output mean(v) per head.
Tokens are permuted live-first so the 672 live queries are contiguous:
scores/exp run only on live columns plus one pinned-zero query column
whose output is exactly mean(v); that column's value is broadcast to the
other 351 masked tokens.

Attention layout: scores/probabilities live as [k_tok, q_tok]; the pv
matmul runs "transposed" (stationary = p chunk, moving = [v | 1]) so each
accumulation step streams only 65 columns and the softmax denominator
lands as column 64 of the per-(head, q-chunk) accumulator.  Normalization
is then a per-partition reciprocal+multiply on DVE, and attnT (needed by
the output projection) is produced by XBAR DMA transposes.
"""
import numpy as np
import ml_dtypes

import concourse.bass as bass
import concourse.mybir as mybir
import concourse.tile as tile
from concourse import bacc
from concourse.bass import ts
from concourse.bass_utils import run_bass_kernel_spmd

B, T, E = 4, 1024, 768
H, D = 12, 64
HG = 6                  # heads per group
GD = HG * D             # 384 head-dims per group
BLOCK_M = 64
LIVE = 672              # tokens with (t % BLOCK_M) % 3 != 0
MASK = T - LIVE         # 352
NQ = LIVE + 1           # live queries + one pinned-zero query (= mean-v row)
SCALE = 1.0 / 8.0       # 1/sqrt(64)
QC = 6                  # q chunks of 128 (last one holds 33 live+pinned cols)

BF16 = mybir.dt.bfloat16
F32 = mybir.dt.float32

_perm = None
_nc = None

TRACE = False
LAST_RES = None


def _perm_live_first():
    t = np.arange(T)
    m = (t % BLOCK_M) % 3 == 0
    return np.concatenate([t[~m], t[m]])


def _build_bass(debug=False):
    nc = bacc.Bacc()
    xT_d = nc.dram_tensor("xT", [E, T], BF16, kind="ExternalInput")
    wT_d = nc.dram_tensor("wT", [E, 3 * GD], BF16, kind="ExternalInput")
    woT_d = nc.dram_tensor("woT", [GD, E], BF16, kind="ExternalInput")
    out_d = nc.dram_tensor("out", [T, E], BF16, kind="ExternalOutput")
    if debug:
        dbg = {nm: nc.dram_tensor(nm, sh, dt, kind="ExternalOutput")
               for nm, sh, dt in (
                   ("dbg_q", [128, 3, NQ], BF16),
                   ("dbg_k", [128, 3, T], BF16),
                   ("dbg_v", [128, 8, HG * (D + 1)], BF16),
                   ("dbg_pp", [128, 768], BF16),
                   ("dbg_acc", [128, QC * (D + 1)], F32),
                   ("dbg_att", [128, QC * 128], BF16),
                   ("dbg_attnT", [128, 3, T], BF16),
               )}

    xT_r = xT_d[:, :].rearrange("(c p) t -> p c t", p=128)
    wT_r = wT_d[:, :].rearrange("(c p) n -> p c n", p=128)

    with tile.TileContext(nc) as tc:
        with tc.tile_pool(name="singles", bufs=1) as S:
            xT_sb = S.tile([128, 6, T], BF16)
            wqk_sb = S.tile([128, 6, 2 * GD], BF16)
            wv_sb = S.tile([128, 6, GD], BF16)
            woT_sb = S.tile([128, 3, E], BF16)
            warm_sb = S.tile([128, 512], BF16)
            qT_sb = S.tile([128, 3, NQ], BF16)
            kT_sb = S.tile([128, 3, T], BF16)
            v_sb = S.tile([128, 8, HG * (D + 1)], BF16)   # per-head [v | 1]
            attnT_sb = S.tile([128, 3, T], BF16)
            dpre_sb = S.tile([1, 1], F32)
            ppb = [S.tile([128, 768], BF16, tag=f"ppb{j}", name=f"ppb{j}")
                   for j in range(5)]

            # ---- input DMAs, chunked so compute starts early
            nc.sync.dma_start(out=xT_sb[:, 0:2, :], in_=xT_r[:, 0:2, :])
            nc.sync.dma_start(out=wv_sb[:, 0:2, :],
                              in_=wT_r[:, 0:2, 2 * GD:3 * GD])
            nc.sync.dma_start(out=xT_sb[:, 2:4, :], in_=xT_r[:, 2:4, :])
            nc.sync.dma_start(out=wv_sb[:, 2:6, :],
                              in_=wT_r[:, 2:6, 2 * GD:3 * GD])
            nc.sync.dma_start(out=xT_sb[:, 4:6, :], in_=xT_r[:, 4:6, :])
            nc.sync.dma_start(out=wqk_sb[:, :, 0:128], in_=wT_r[:, :, 0:128])
            nc.sync.dma_start(out=wqk_sb[:, :, GD:GD + 128],
                              in_=wT_r[:, :, GD:GD + 128])
            nc.sync.dma_start(out=wqk_sb[:, :, 128:GD],
                              in_=wT_r[:, :, 128:GD])
            nc.sync.dma_start(out=wqk_sb[:, :, GD + 128:2 * GD],
                              in_=wT_r[:, :, GD + 128:2 * GD])
            nc.sync.dma_start(out=woT_sb,
                              in_=woT_d[:, :].rearrange("(c p) n -> p c n", p=128))

            nc.vector.memset(warm_sb, 0.5)
            for j in range(5):
                # pad cols so the last q-chunk's [128,128] stationary reads
                # defined small values (keeps denominators finite)
                nc.vector.memset(ppb[j][:, NQ:768], 1e-10)
            v_ones = v_sb[:, :, :].rearrange("p a (h e) -> p a h e", e=D + 1)[:, :, :, D:D + 1]
            nc.vector.memset(v_ones, 1.0)
            # pinned-zero query column (mean-v row for masked tokens)
            nc.vector.memset(qT_sb[:, :, LIVE:NQ], 0.0)
            # preload the exp table while DMAs run
            nc.scalar.activation(dpre_sb, warm_sb[0:1, 0:1],
                                 mybir.ActivationFunctionType.Exp)

            # ---- Phase V + QK + attention.  V projects through 6 psum
            # banks (tts 0-5 pass-major, tts 6-7 tt-major on reused banks)
            # so the first qk slices can grab banks while V still runs.
            from contextlib import ExitStack
            with tc.tile_pool(name="ps_qk", bufs=2, space="PSUM") as ps_qk, \
                 tc.tile_pool(name="att", bufs=2) as att_pool, \
                 tc.tile_pool(name="rq", bufs=2) as rq_pool, \
                 ExitStack() as vstack:
                vp = vstack.enter_context(
                    tc.tile_pool(name="vps", bufs=1, space="PSUM"))

                ppi = 0

                def qk_slice(c, which, s0, s1, j):
                    # one 1-bank slice of the q or k projection for pair c
                    qp = ps_qk.tile([128, 512], F32, tag="qk",
                                    name=f"qk{c}_{j}")
                    wof = c * 128 if which == "q" else GD + c * 128
                    dstT = qT_sb if which == "q" else kT_sb
                    for ek in range(6):
                        nc.tensor.matmul(qp[:, 0:s1 - s0],
                                         wqk_sb[:, ek, wof:wof + 128],
                                         xT_sb[:, ek, s0:s1],
                                         start=(ek == 0), stop=(ek == 5))
                    nc.vector.tensor_copy(dstT[:, c, s0:s1], qp[:, 0:s1 - s0])

                vt = [vp.tile([128, GD], F32, tag=f"v{tt % 6}", name=f"vt{tt}")
                      for tt in range(6)]
                for i in range(5):
                    nc.tensor.matmul(vt[0], warm_sb[:, 0:128], warm_sb[:, 0:384],
                                     start=True, stop=True)

                def vcopy(tt, t):
                    dst = v_sb[:, tt, :].rearrange(
                        "p (h e) -> p h e", e=D + 1)[:, :, 0:D]
                    src = t[:, :].rearrange("p (h d) -> p h d", d=D)
                    if tt % 2 == 0:
                        nc.vector.tensor_copy(dst, src)
                    else:
                        nc.scalar.copy(dst, src)

                for gi, grp in enumerate(((0, 1), (2, 3), (4,), (5,))):
                    for tt in range(6):
                        for ek in grp:
                            nc.tensor.matmul(vt[tt],
                                             xT_sb[:, ek, ts(tt, 128)],
                                             wv_sb[:, ek, :],
                                             start=(ek == 0), stop=(ek == 5))
                        if grp[-1] == 5:
                            vcopy(tt, vt[tt])
                    if gi == 1:
                        qk_slice(0, "q", 0, 512, 0)
                    elif gi == 2:
                        qk_slice(0, "k", 0, 512, 1)
                for tt in (6, 7):
                    t2 = vp.tile([128, GD], F32, tag=f"v{tt % 6}",
                                 name=f"vt{tt}")
                    for ek in range(6):
                        nc.tensor.matmul(t2, xT_sb[:, ek, ts(tt, 128)],
                                         wv_sb[:, ek, :],
                                         start=(ek == 0), stop=(ek == 5))
                    vcopy(tt, t2)
                    if tt == 6:
                        qk_slice(0, "q", 512, LIVE, 2)
                vstack.close()
                ps_s = vstack.enter_context(
                    tc.tile_pool(name="ps_s", bufs=2, space="PSUM"))
                ps_acc = vstack.enter_context(
                    tc.tile_pool(name="ps_acc", bufs=1, space="PSUM"))
                for c in range(3):
                    accs = [ps_acc.tile([128, QC * (D + 1)], F32,
                                        tag=f"acc{i}", name=f"acc{c}_{i}")
                            for i in range(2)]

                    def pv(kt, i, pp):
                        h = 2 * c + i
                        vh = v_sb[:, kt, h * (D + 1):(h + 1) * (D + 1)]
                        for qc in range(QC):
                            nc.tensor.matmul(
                                accs[i][:, qc * (D + 1):(qc + 1) * (D + 1)],
                                pp[:, qc * 128:(qc + 1) * 128],
                                vh,
                                # exactly one start=True per psum bank: it
                                # clears the whole bank's has_written, so the
                                # other regions' first writes (start=False on
                                # cleared bits) overwrite rather than add
                                start=(kt == 0 and qc == 0), stop=(kt == 7),
                                skip_group_check=True)

                    pend = []
                    for kt in range(8):
                        for i in range(2):      # head 2c+i
                            po = i * 64
                            kh = kT_sb[po:po + 64, c, ts(kt, 128)]
                            qh = qT_sb[po:po + 64, c, :]
                            sp = ps_s.tile([128, T], F32, tag="s",
                                           name=f"s{c}_{kt}_{i}")
                            nc.tensor.matmul(sp[:, 0:512], kh, qh[:, 0:512],
                                             start=True, stop=True)
                            nc.tensor.matmul(sp[:, 512:NQ], kh, qh[:, 512:NQ],
                                             start=True, stop=True)
                            pp = ppb[ppi % 5]
                            ppi += 1
                            nc.scalar.activation(pp[:, 0:NQ], sp[:, 0:NQ],
                                                 mybir.ActivationFunctionType.Exp,
                                                 scale=SCALE)
                            if debug and c == 0 and kt == 0 and i == 0:
                                nc.sync.dma_start(out=dbg["dbg_pp"][:, :], in_=pp)
                            pend.append((kt, i, pp))
                        # pv runs one kt behind exp so no PE instruction ever
                        # sits in the wait queue blocking later scores
                        while len(pend) > 3:
                            pv(*pend.pop(0))
                        if kt == 0 and c == 0:
                            qk_slice(0, "k", 512, T, 3)
                        if 2 <= kt <= 5 and c < 2:
                            j = kt - 2
                            which, s0, s1 = (("q", 0, 512), ("k", 0, 512),
                                             ("q", 512, LIVE), ("k", 512, T))[j]
                            qk_slice(c + 1, which, s0, s1, j)
                    # ---- normalize: per-partition recip + strided multiply
                    att = att_pool.tile([128, QC * 128], BF16, tag="att")
                    def norm(i):
                        rq = rq_pool.tile([128, QC], F32, tag=f"rq{i}")
                        den = bass.AP(tensor=accs[i].tensor, offset=accs[i].offset + D,
                                      ap=[list(accs[i].ap[0])] + [[D + 1, QC]])
                        nc.vector.reciprocal(rq, den)
                        src = bass.AP(tensor=accs[i].tensor, offset=accs[i].offset,
                                      ap=[list(accs[i].ap[0])] + [[D + 1, QC], [1, D]])
                        sca = bass.AP(tensor=rq.tensor, offset=rq.offset,
                                      ap=[list(rq.ap[0])] + [[1, QC], [0, D]])
                        dst = bass.AP(tensor=att.tensor, offset=att.offset + i * 64,
                                      ap=[list(att.ap[0])] + [[128, QC], [1, D]])
                        nc.vector.tensor_mul(dst, src, sca)
                    while pend:
                        kt_, i_, pp_ = pend.pop(0)
                        pv(kt_, i_, pp_)
                        if not any(e[1] == i_ for e in pend):
                            norm(i_)
                    # ---- transpose -> attnT [dims, tok].  c0/c1 ride the
                    # idle DMA engines (XBAR); c2 is on the critical tail, so
                    # use PE transposes + an ACT copy (no DMA latency) into a
                    # separate tile (keeps out-proj deps on c0/c1 precise)
                    o = attnT_sb[:, c, :]
                    if c < 2:
                        o3 = bass.AP(tensor=o.tensor, offset=o.offset,
                                     ap=[list(o.ap[0])] + [[128, QC], [1, 128]])
                        nc.sync.dma_start_transpose(o3, att[:, :])
                    else:
                        for h0 in (0, 3):
                            oh = bass.AP(tensor=o.tensor,
                                         offset=o.offset + h0 * 128,
                                         ap=[list(o.ap[0])] + [[128, 3], [1, 128]])
                            nc.sync.dma_start_transpose(
                                oh, att[:, h0 * 128:(h0 + 3) * 128])
                    if debug and c == 0:
                        accc = S.tile([128, QC * (D + 1)], F32, name="accc")
                        nc.vector.tensor_copy(accc, accs[0])
                        nc.sync.dma_start(out=dbg["dbg_acc"][:, :], in_=accc)
                        nc.sync.dma_start(out=dbg["dbg_att"][:, :], in_=att)

                if debug:
                    for nm, t in (("dbg_q", qT_sb), ("dbg_k", kT_sb),
                                  ("dbg_v", v_sb), ("dbg_attnT", attnT_sb)):
                        nc.sync.dma_start(out=dbg[nm][:, :, :], in_=t[:, :, :])

            # ---- Phase OUT: output projection (partial over group's dims);
            # one whole-tile staging copy per tt, alternating DVE/ACT
            with tc.tile_pool(name="ops", bufs=4, space="PSUM") as op, \
                 tc.tile_pool(name="ob", bufs=4) as obp:
                for tt in range(6):
                    ps = op.tile([128, E], F32, tag="o")
                    for s0, s1 in ((0, 512), (512, E)):
                        for c3 in range(3):
                            nc.tensor.matmul(ps[:, s0:s1],
                                             attnT_sb[:, c3, ts(tt, 128)],
                                             woT_sb[:, c3, s0:s1],
                                             start=(c3 == 0), stop=(c3 == 2))
                    if tt >= 4:
                        ob1 = obp.tile([128, E], BF16, tag="ob1",
                                       name=f"ob1_{tt}")
                        if tt == 5:
                            # rows 673:768 are masked duplicates the host
                            # rebuilds; ship only the 33 rows that matter
                            nc.scalar.copy(ob1[0:33, :], ps[0:33, :])
                            nc.sync.dma_start(out=out_d[640:673, :],
                                              in_=ob1[0:33, :])
                        else:
                            nc.vector.tensor_copy(ob1, ps)
                            nc.sync.dma_start(out=out_d[ts(tt, 128), :],
                                              in_=ob1)
                    elif tt % 2 == 0:
                        ob = obp.tile([128, 2, E], BF16, tag="ob")
                        nc.vector.tensor_copy(ob[:, 0, :], ps)
                    else:
                        nc.scalar.copy(ob[:, 1, :], ps)
                        nc.sync.dma_start(
                            out=out_d[(tt - 1) * 128:(tt + 1) * 128, :]
                            .rearrange("(j p) e -> p j e", p=128),
                            in_=ob)

    nc.finalize()
    return nc


def _get_bass():
    global _nc
    if _nc is None:
        _nc = _build_bass()
    return _nc


def kernel(x, idx, struct_embed, w_qkv, w_out, b_out):
    global _perm
    if _perm is None:
        _perm = _perm_live_first()
    perm = _perm

    x = np.asarray(x, dtype=np.float32)
    idx = np.asarray(idx)
    struct_embed = np.asarray(struct_embed, dtype=np.float32)
    w_qkv = np.asarray(w_qkv, dtype=np.float32)
    w_out = np.asarray(w_out, dtype=np.float32)
    b_out = np.asarray(b_out, dtype=np.float32)

    sid = ((idx == 1) * 1 + (idx == 2) * 2 + (idx == 3) * 3)  # [B,T]
    xs = x + struct_embed[sid]                                # fp32 pre-add

    bf = ml_dtypes.bfloat16
    in_maps = []
    for core in range(8):
        b, g = core // 2, core % 2
        wg = np.concatenate([w_qkv[g * GD:(g + 1) * GD],
                             w_qkv[E + g * GD:E + (g + 1) * GD],
                             w_qkv[2 * E + g * GD:2 * E + (g + 1) * GD]],
                            axis=0)  # [3GD, E]
        in_maps.append({
            "xT": np.ascontiguousarray(xs[b].T[:, perm]).astype(bf),
            "wT": np.ascontiguousarray(wg.T).astype(bf),
            "woT": np.ascontiguousarray(w_out[:, g * GD:(g + 1) * GD].T).astype(bf),
        })

    res = run_bass_kernel_spmd(_get_bass(), in_maps, core_ids=list(range(8)),
                               trace=TRACE)
    if TRACE:
        global LAST_RES
        LAST_RES = res

    inv = np.empty(T, dtype=np.int64)
    inv[perm] = np.arange(T)
    out = np.empty((B, T, E), dtype=np.float32)
    for b in range(B):
        acc = (res.results[2 * b]["out"].astype(np.float32)
               + res.results[2 * b + 1]["out"].astype(np.float32))
        # device computes permuted rows 0:NQ; all masked-token rows are
        # identical to row LIVE (the pinned-zero query = mean-v output)
        acc[NQ:] = acc[LIVE]
        out[b] = acc[inv] + b_out[None, :]
    return out


# revision 86
# speedup vs baseline: 1.0019x; 1.0012x over previous
"""Trainium2 Bass kernel for nn_MultiHeadAttention_8074538516581.

Sharding: 8 cores = batch(4) x head-group(2 groups of 6 heads).
Each core computes, for its (b, g): qkv slice projection (bf16 matmuls,
fp32 psum accum; the struct-embed term is pre-added into x on the host in
fp32), per-head attention with the reference's exact semantics (q/k rounded
to bf16, shift-free softmax -- the row-max subtraction cancels in the
normalization; the [-30,30] logit clip and the 1e5/1e-10 guards are
provably inactive here), and the partial output projection over its 384
head-dims.  Host sums the two head-group partials per batch and adds b_out.

Token permutation: queries with (t % 64) % 3 == 0 are zeroed by the
reference's load mask, making their attention output mean(v) per head.
Tokens are permuted live-first so the 672 live queries are contiguous:
scores/exp run only on live columns plus one pinned-zero query column
whose output is exactly mean(v); that column's value is broadcast to the
other 351 masked tokens.

Attention layout: scores/probabilities live as [k_tok, q_tok]; the pv
matmul runs "transposed" (stationary = p chunk, moving = [v | 1]) so each
accumulation step streams only 65 columns and the softmax denominator
lands as column 64 of the per-(head, q-chunk) accumulator.  Normalization
is then a per-partition reciprocal+multiply on DVE, and attnT (needed by
the output projection) is produced by XBAR DMA transposes.
"""
import numpy as np
import ml_dtypes

import concourse.bass as bass
import concourse.mybir as mybir
import concourse.tile as tile
from concourse import bacc
from concourse.bass import ts
from concourse.bass_utils import run_bass_kernel_spmd

B, T, E = 4, 1024, 768
H, D = 12, 64
HG = 6                  # heads per group
GD = HG * D             # 384 head-dims per group
BLOCK_M = 64
LIVE = 672              # tokens with (t % BLOCK_M) % 3 != 0
MASK = T - LIVE         # 352
NQ = LIVE + 1           # live queries + one pinned-zero query (= mean-v row)
SCALE = 1.0 / 8.0       # 1/sqrt(64)
QC = 6                  # q chunks of 128 (last one holds 33 live+pinned cols)

BF16 = mybir.dt.bfloat16
F32 = mybir.dt.float32

_perm = None
_nc = None

TRACE = False
LAST_RES = None


def _perm_live_first():
    t = np.arange(T)
    m = (t % BLOCK_M) % 3 == 0
    return np.concatenate([t[~m], t[m]])


def _build_bass(debug=False):
    nc = bacc.Bacc()
    xT_d = nc.dram_tensor("xT", [E, T], BF16, kind="ExternalInput")
    wT_d = nc.dram_tensor("wT", [E, 3 * GD], BF16, kind="ExternalInput")
    woT_d = nc.dram_tensor("woT", [GD, E], BF16, kind="ExternalInput")
    out_d = nc.dram_tensor("out", [T, E], BF16, kind="ExternalOutput")
    if debug:
        dbg = {nm: nc.dram_tensor(nm, sh, dt, kind="ExternalOutput")
               for nm, sh, dt in (
                   ("dbg_q", [128, 3, NQ], BF16),
                   ("dbg_k", [128, 3, T], BF16),
                   ("dbg_v", [128, 8, HG * (D + 1)], BF16),
                   ("dbg_pp", [128, 768], BF16),
                   ("dbg_acc", [128, QC * (D + 1)], F32),
                   ("dbg_att", [128, QC * 128], BF16),
                   ("dbg_attnT", [128, 3, T], BF16),
               )}

    xT_r = xT_d[:, :].rearrange("(c p) t -> p c t", p=128)
    wT_r = wT_d[:, :].rearrange("(c p) n -> p c n", p=128)

    with tile.TileContext(nc) as tc:
        with tc.tile_pool(name="singles", bufs=1) as S:
            xT_sb = S.tile([128, 6, T], BF16)
            wqk_sb = S.tile([128, 6, 2 * GD], BF16)
            wv_sb = S.tile([128, 6, GD], BF16)
            woT_sb = S.tile([128, 3, E], BF16)
            warm_sb = S.tile([128, 512], BF16)
            qT_sb = S.tile([128, 3, NQ], BF16)
            kT_sb = S.tile([128, 3, T], BF16)
            v_sb = S.tile([128, 8, HG * (D + 1)], BF16)   # per-head [v | 1]
            attnT_sb = S.tile([128, 3, T], BF16)
            dpre_sb = S.tile([1, 1], F32)
            ppb = [S.tile([128, 768], BF16, tag=f"ppb{j}", name=f"ppb{j}")
                   for j in range(5)]

            # ---- input DMAs, chunked so compute starts early
            nc.sync.dma_start(out=xT_sb[:, 0:2, :], in_=xT_r[:, 0:2, :])
            nc.sync.dma_start(out=wv_sb[:, 0:2, :],
                              in_=wT_r[:, 0:2, 2 * GD:3 * GD])
            nc.sync.dma_start(out=xT_sb[:, 2:4, :], in_=xT_r[:, 2:4, :])
            nc.sync.dma_start(out=wv_sb[:, 2:6, :],
                              in_=wT_r[:, 2:6, 2 * GD:3 * GD])
            nc.sync.dma_start(out=xT_sb[:, 4:6, :], in_=xT_r[:, 4:6, :])
            nc.sync.dma_start(out=wqk_sb[:, :, 0:128], in_=wT_r[:, :, 0:128])
            nc.sync.dma_start(out=wqk_sb[:, :, GD:GD + 128],
                              in_=wT_r[:, :, GD:GD + 128])
            nc.sync.dma_start(out=wqk_sb[:, :, 128:GD],
                              in_=wT_r[:, :, 128:GD])
            nc.sync.dma_start(out=wqk_sb[:, :, GD + 128:2 * GD],
                              in_=wT_r[:, :, GD + 128:2 * GD])
            nc.sync.dma_start(out=woT_sb,
                              in_=woT_d[:, :].rearrange("(c p) n -> p c n", p=128))

            nc.vector.memset(warm_sb, 0.5)
            for j in range(5):
                # pad cols so the last q-chunk's [128,128] stationary reads
                # defined small values (keeps denominators finite)
                nc.vector.memset(ppb[j][:, NQ:768], 1e-10)
            v_ones = v_sb[:, :, :].rearrange("p a (h e) -> p a h e", e=D + 1)[:, :, :, D:D + 1]
            nc.vector.memset(v_ones, 1.0)
            # pinned-zero query column (mean-v row for masked tokens)
            nc.vector.memset(qT_sb[:, :, LIVE:NQ], 0.0)
            # preload the exp table while DMAs run
            nc.scalar.activation(dpre_sb, warm_sb[0:1, 0:1],
                                 mybir.ActivationFunctionType.Exp)

            # ---- Phase V + QK + attention.  V projects through 6 psum
            # banks (tts 0-5 pass-major, tts 6-7 tt-major on reused banks)
            # so the first qk slices can grab banks while V still runs.
            from contextlib import ExitStack
            with tc.tile_pool(name="ps_qk", bufs=2, space="PSUM") as ps_qk, \
                 tc.tile_pool(name="att", bufs=2) as att_pool, \
                 tc.tile_pool(name="rq", bufs=2) as rq_pool, \
                 ExitStack() as vstack:
                vp = vstack.enter_context(
                    tc.tile_pool(name="vps", bufs=1, space="PSUM"))

                ppi = 0

                def qk_slice(c, which, s0, s1, j):
                    # one 1-bank slice of the q or k projection for pair c
                    qp = ps_qk.tile([128, 512], F32, tag="qk",
                                    name=f"qk{c}_{j}")
                    wof = c * 128 if which == "q" else GD + c * 128
                    dstT = qT_sb if which == "q" else kT_sb
                    for ek in range(6):
                        nc.tensor.matmul(qp[:, 0:s1 - s0],
                                         wqk_sb[:, ek, wof:wof + 128],
                                         xT_sb[:, ek, s0:s1],
                                         start=(ek == 0), stop=(ek == 5))
                    nc.vector.tensor_copy(dstT[:, c, s0:s1], qp[:, 0:s1 - s0])

                vt = [vp.tile([128, GD], F32, tag=f"v{tt % 6}", name=f"vt{tt}")
                      for tt in range(6)]
                for i in range(5):
                    nc.tensor.matmul(vt[0], warm_sb[:, 0:128], warm_sb[:, 0:384],
                                     start=True, stop=True)

                def vcopy(tt, t):
                    dst = v_sb[:, tt, :].rearrange(
                        "p (h e) -> p h e", e=D + 1)[:, :, 0:D]
                    src = t[:, :].rearrange("p (h d) -> p h d", d=D)
                    nc.scalar.copy(dst, src)

                for gi, grp in enumerate(((0, 1), (2, 3), (4,), (5,))):
                    for tt in range(6):
                        for ek in grp:
                            nc.tensor.matmul(vt[tt],
                                             xT_sb[:, ek, ts(tt, 128)],
                                             wv_sb[:, ek, :],
                                             start=(ek == 0), stop=(ek == 5))
                        if grp[-1] == 5:
                            vcopy(tt, vt[tt])
                    if gi == 1:
                        qk_slice(0, "q", 0, 512, 0)
                    elif gi == 2:
                        qk_slice(0, "k", 0, 512, 1)
                for tt in (6, 7):
                    t2 = vp.tile([128, GD], F32, tag=f"v{tt % 6}",
                                 name=f"vt{tt}")
                    for ek in range(6):
                        nc.tensor.matmul(t2, xT_sb[:, ek, ts(tt, 128)],
                                         wv_sb[:, ek, :],
                                         start=(ek == 0), stop=(ek == 5))
                    vcopy(tt, t2)
                    if tt == 6:
                        qk_slice(0, "q", 512, LIVE, 2)
                vstack.close()
                ps_s = vstack.enter_context(
                    tc.tile_pool(name="ps_s", bufs=2, space="PSUM"))
                ps_acc = vstack.enter_context(
                    tc.tile_pool(name="ps_acc", bufs=1, space="PSUM"))
                for c in range(3):
                    accs = [ps_acc.tile([128, QC * (D + 1)], F32,
                                        tag=f"acc{i}", name=f"acc{c}_{i}")
                            for i in range(2)]

                    def pv(kt, i, pp):
                        h = 2 * c + i
                        vh = v_sb[:, kt, h * (D + 1):(h + 1) * (D + 1)]
                        for qc in range(QC):
                            nc.tensor.matmul(
                                accs[i][:, qc * (D + 1):(qc + 1) * (D + 1)],
                                pp[:, qc * 128:(qc + 1) * 128],
                                vh,
                                # exactly one start=True per psum bank: it
                                # clears the whole bank's has_written, so the
                                # other regions' first writes (start=False on
                                # cleared bits) overwrite rather than add
                                start=(kt == 0 and qc == 0), stop=(kt == 7),
                                skip_group_check=True)

                    pend = []
                    for kt in range(8):
                        for i in range(2):      # head 2c+i
                            po = i * 64
                            kh = kT_sb[po:po + 64, c, ts(kt, 128)]
                            qh = qT_sb[po:po + 64, c, :]
                            sp = ps_s.tile([128, T], F32, tag="s",
                                           name=f"s{c}_{kt}_{i}")
                            nc.tensor.matmul(sp[:, 0:512], kh, qh[:, 0:512],
                                             start=True, stop=True)
                            nc.tensor.matmul(sp[:, 512:NQ], kh, qh[:, 512:NQ],
                                             start=True, stop=True)
                            pp = ppb[ppi % 5]
                            ppi += 1
                            nc.scalar.activation(pp[:, 0:NQ], sp[:, 0:NQ],
                                                 mybir.ActivationFunctionType.Exp,
                                                 scale=SCALE)
                            if debug and c == 0 and kt == 0 and i == 0:
                                nc.sync.dma_start(out=dbg["dbg_pp"][:, :], in_=pp)
                            pend.append((kt, i, pp))
                        # pv runs one kt behind exp so no PE instruction ever
                        # sits in the wait queue blocking later scores
                        while len(pend) > 3:
                            pv(*pend.pop(0))
                        if kt == 0 and c == 0:
                            qk_slice(0, "k", 512, T, 3)
                        if 2 <= kt <= 5 and c < 2:
                            j = kt - 2
                            which, s0, s1 = (("q", 0, 512), ("k", 0, 512),
                                             ("q", 512, LIVE), ("k", 512, T))[j]
                            qk_slice(c + 1, which, s0, s1, j)
                    # ---- normalize: per-partition recip + strided multiply
                    att = att_pool.tile([128, QC * 128], BF16, tag="att")
                    def norm(i):
                        rq = rq_pool.tile([128, QC], F32, tag=f"rq{i}")
                        den = bass.AP(tensor=accs[i].tensor, offset=accs[i].offset + D,
                                      ap=[list(accs[i].ap[0])] + [[D + 1, QC]])
                        nc.vector.reciprocal(rq, den)
                        src = bass.AP(tensor=accs[i].tensor, offset=accs[i].offset,
                                      ap=[list(accs[i].ap[0])] + [[D + 1, QC], [1, D]])
                        sca = bass.AP(tensor=rq.tensor, offset=rq.offset,
                                      ap=[list(rq.ap[0])] + [[1, QC], [0, D]])
                        dst = bass.AP(tensor=att.tensor, offset=att.offset + i * 64,
                                      ap=[list(att.ap[0])] + [[128, QC], [1, D]])
                        nc.vector.tensor_mul(dst, src, sca)
                    while pend:
                        kt_, i_, pp_ = pend.pop(0)
                        pv(kt_, i_, pp_)
                        if not any(e[1] == i_ for e in pend):
                            norm(i_)
                    # ---- transpose -> attnT [dims, tok].  c0/c1 ride the
                    # idle DMA engines (XBAR); c2 is on the critical tail, so
                    # use PE transposes + an ACT copy (no DMA latency) into a
                    # separate tile (keeps out-proj deps on c0/c1 precise)
                    o = attnT_sb[:, c, :]
                    if c < 2:
                        o3 = bass.AP(tensor=o.tensor, offset=o.offset,
                                     ap=[list(o.ap[0])] + [[128, QC], [1, 128]])
                        nc.sync.dma_start_transpose(o3, att[:, :])
                    else:
                        for h0 in (0, 3):
                            oh = bass.AP(tensor=o.tensor,
                                         offset=o.offset + h0 * 128,
                                         ap=[list(o.ap[0])] + [[128, 3], [1, 128]])
                            nc.sync.dma_start_transpose(
                                oh, att[:, h0 * 128:(h0 + 3) * 128])
                    if debug and c == 0:
                        accc = S.tile([128, QC * (D + 1)], F32, name="accc")
                        nc.vector.tensor_copy(accc, accs[0])
                        nc.sync.dma_start(out=dbg["dbg_acc"][:, :], in_=accc)
                        nc.sync.dma_start(out=dbg["dbg_att"][:, :], in_=att)

                if debug:
                    for nm, t in (("dbg_q", qT_sb), ("dbg_k", kT_sb),
                                  ("dbg_v", v_sb), ("dbg_attnT", attnT_sb)):
                        nc.sync.dma_start(out=dbg[nm][:, :, :], in_=t[:, :, :])

            # ---- Phase OUT: output projection (partial over group's dims);
            # one whole-tile staging copy per tt, alternating DVE/ACT
            with tc.tile_pool(name="ops", bufs=4, space="PSUM") as op, \
                 tc.tile_pool(name="ob", bufs=4) as obp:
                for tt in range(6):
                    ps = op.tile([128, E], F32, tag="o")
                    for s0, s1 in ((0, 512), (512, E)):
                        for c3 in range(3):
                            nc.tensor.matmul(ps[:, s0:s1],
                                             attnT_sb[:, c3, ts(tt, 128)],
                                             woT_sb[:, c3, s0:s1],
                                             start=(c3 == 0), stop=(c3 == 2))
                    if tt >= 4:
                        ob1 = obp.tile([128, E], BF16, tag="ob1",
                                       name=f"ob1_{tt}")
                        if tt == 5:
                            # rows 673:768 are masked duplicates the host
                            # rebuilds; ship only the 33 rows that matter
                            nc.scalar.copy(ob1[0:33, :], ps[0:33, :])
                            nc.sync.dma_start(out=out_d[640:673, :],
                                              in_=ob1[0:33, :])
                        else:
                            nc.vector.tensor_copy(ob1, ps)
                            nc.sync.dma_start(out=out_d[ts(tt, 128), :],
                                              in_=ob1)
                    elif tt % 2 == 0:
                        ob = obp.tile([128, 2, E], BF16, tag="ob")
                        nc.vector.tensor_copy(ob[:, 0, :], ps)
                    else:
                        nc.scalar.copy(ob[:, 1, :], ps)
                        nc.sync.dma_start(
                            out=out_d[(tt - 1) * 128:(tt + 1) * 128, :]
                            .rearrange("(j p) e -> p j e", p=128),
                            in_=ob)

    nc.finalize()
    return nc


def _get_bass():
    global _nc
    if _nc is None:
        _nc = _build_bass()
    return _nc


def kernel(x, idx, struct_embed, w_qkv, w_out, b_out):
    global _perm
    if _perm is None:
        _perm = _perm_live_first()
    perm = _perm

    x = np.asarray(x, dtype=np.float32)
    idx = np.asarray(idx)
    struct_embed = np.asarray(struct_embed, dtype=np.float32)
    w_qkv = np.asarray(w_qkv, dtype=np.float32)
    w_out = np.asarray(w_out, dtype=np.float32)
    b_out = np.asarray(b_out, dtype=np.float32)

    sid = ((idx == 1) * 1 + (idx == 2) * 2 + (idx == 3) * 3)  # [B,T]
    xs = x + struct_embed[sid]                                # fp32 pre-add

    bf = ml_dtypes.bfloat16
    in_maps = []
    for core in range(8):
        b, g = core // 2, core % 2
        wg = np.concatenate([w_qkv[g * GD:(g + 1) * GD],
                             w_qkv[E + g * GD:E + (g + 1) * GD],
                             w_qkv[2 * E + g * GD:2 * E + (g + 1) * GD]],
                            axis=0)  # [3GD, E]
        in_maps.append({
            "xT": np.ascontiguousarray(xs[b].T[:, perm]).astype(bf),
            "wT": np.ascontiguousarray(wg.T).astype(bf),
            "woT": np.ascontiguousarray(w_out[:, g * GD:(g + 1) * GD].T).astype(bf),
        })

    res = run_bass_kernel_spmd(_get_bass(), in_maps, core_ids=list(range(8)),
                               trace=TRACE)
    if TRACE:
        global LAST_RES
        LAST_RES = res

    inv = np.empty(T, dtype=np.int64)
    inv[perm] = np.arange(T)
    out = np.empty((B, T, E), dtype=np.float32)
    for b in range(B):
        acc = (res.results[2 * b]["out"].astype(np.float32)
               + res.results[2 * b + 1]["out"].astype(np.float32))
        # device computes permuted rows 0:NQ; all masked-token rows are
        # identical to row LIVE (the pinned-zero query = mean-v output)
        acc[NQ:] = acc[LIVE]
        out[b] = acc[inv] + b_out[None, :]
    return out
